# revision 2
# baseline (speedup 1.0000x reference)
import numpy as np

# nn_Attention_7765300871328 (sparse_attention) — Trainium (NeuronCore) kernel.
# Self-contained: takes FULL inputs, returns FULL output [1, T, HIDDEN].
# Strategy: the whole fused module (projections + RoPE + compressed attention
# + topk block selection + block-sparse attention + sliding-window attention
# + gated fusion + output projection) is compiled as ONE jitted graph for a
# NeuronCore via the Neuron PJRT backend. Heavy matmuls run in bf16 (PE runs
# bf16 at 2x fp32 rate); the compressed-attention/top-k selection path stays
# fp32 so block selection matches the reference's lax.top_k semantics.

T = 2048; HIDDEN = 2048; HQ = 32; G = 2; D = 64
KERNEL = 32; STRIDE = 16; BLOCK = 64; TOPK = 16
INIT_BLOCKS = 1; LOCAL_BLOCKS = 2; WINDOW = 512
ROPE_BASE = 10000.0
NEG = np.float32(-1e30)

C = (T - KERNEL) // STRIDE + 1
NB = T // BLOCK

_STATE = {}


def _host_consts():
    starts = np.arange(C) * STRIDE
    win_idx = (starts[:, None] + np.arange(KERNEL)[None, :]).astype(np.int32)
    cmask = (np.arange(T)[:, None] >= (starts + KERNEL - 1)[None, :])          # [T, C]
    valid = cmask.any(-1)                                                      # [T]
    bstart = np.arange(NB) * BLOCK
    overlap = ((starts[:, None] < bstart[None, :] + BLOCK) &
               (starts[:, None] + KERNEL > bstart[None, :])).astype(np.float32)  # [C, NB]
    qblock = np.arange(T) // BLOCK
    b = np.arange(NB)
    causal_b = b[None, :] <= qblock[:, None]                                   # [T, NB]
    forced = (b[None, :] < INIT_BLOCKS) | (
        ((qblock[:, None] - b[None, :]) < LOCAL_BLOCKS) & causal_b)
    half = D // 2
    inv = 1.0 / (ROPE_BASE ** (np.arange(half, dtype=np.float64) / half))
    freqs = np.arange(T, dtype=np.float64)[:, None] * inv[None, :]
    cos = np.cos(freqs).astype(np.float32)                                     # [T, half]
    sin = np.sin(freqs).astype(np.float32)
    return dict(win_idx=win_idx, cmask=cmask, valid=valid, overlap=overlap,
                causal_b=causal_b, forced=forced, cos=cos, sin=sin)


def _build():
    import jax, jax.numpy as jnp
    cst = {k: v for k, v in _host_consts().items()}
    bf16 = jnp.bfloat16
    f32 = jnp.float32

    def rope(x, cos, sin):
        # x: [T, H, D]
        half = D // 2
        x1, x2 = x[..., :half], x[..., half:]
        c = cos[:, None, :]
        s = sin[:, None, :]
        return jnp.concatenate([x1 * c - x2 * s, x2 * c + x1 * s], axis=-1)

    def fwd(hidden_states, Wq, Wk, Wv, Wo, Wgate, compress_key, compress_value):
        x = hidden_states[0]                                   # [T, HIDDEN]
        scale = np.float32(1.0 / np.sqrt(D))
        xb = x.astype(bf16)
        q = (xb @ Wq.T.astype(bf16)).astype(f32).reshape(T, HQ, D)
        k = (xb @ Wk.T.astype(bf16)).astype(f32).reshape(T, G, D)
        v = (xb @ Wv.T.astype(bf16)).astype(f32).reshape(T, G, D)
        gate = jax.nn.sigmoid((xb @ Wgate.T.astype(bf16)).astype(f32))  # [T, 3]
        cos = jnp.asarray(cst["cos"]); sin = jnp.asarray(cst["sin"])
        q = rope(q, cos, sin)
        k = rope(k, cos, sin)

        # --- branch 1: compressed attention (fp32 — feeds top-k selection) ---
        kw = k[cst["win_idx"]].transpose(2, 0, 1, 3).reshape(G, C, KERNEL * D)
        vw = v[cst["win_idx"]].transpose(2, 0, 1, 3).reshape(G, C, KERNEL * D)
        ck = jnp.einsum('gcf,gfd->cgd', kw, compress_key)      # [C, G, D]
        cv = jnp.einsum('gcf,gfd->cgd', vw, compress_value)

        qg = q.reshape(T, G, HQ // G, D)
        cs = jnp.einsum('tghd,cgd->ghtc', qg, ck) * scale      # [G, grp, T, C]
        cmask = jnp.asarray(cst["cmask"])
        cs = jnp.where(cmask[None, None], cs, NEG)
        p = jax.nn.softmax(cs, axis=-1)
        valid = jnp.asarray(cst["valid"])
        p = jnp.where(valid[None, None, :, None], p, 0.0)
        comp_out = jnp.einsum('ghtc,cgd->tghd', p, cv).reshape(T, HQ, D)

        # --- topk block selection (fp32) ---
        score = jnp.einsum('ghtc,cb->gtb', p, jnp.asarray(cst["overlap"]))  # [G,T,NB]
        causal_b = jnp.asarray(cst["causal_b"])
        forced = jnp.asarray(cst["forced"])
        score = jnp.where(forced[None], jnp.inf, score)
        score = jnp.where(causal_b[None], score, -jnp.inf)
        _, topk_idx = jax.lax.top_k(score, TOPK)               # [G, T, TOPK]
        tk_valid = jnp.take_along_axis(score, topk_idx, axis=-1) > -jnp.inf
        bb = jnp.arange(NB)
        sel = ((topk_idx[..., None] == bb) & tk_valid[..., None]).any(axis=2)  # [G,T,NB]

        # --- shared dense scores for branches 2 & 3 (bf16 matmul) ---
        qb = qg.astype(bf16)
        kb = k.astype(bf16)
        vb = v.astype(bf16)
        s = jnp.einsum('tghd,sgd->ghts', qb, kb,
                       preferred_element_type=f32) * scale     # [G, grp, T, Tk]

        ar = jnp.arange(T)
        causal = ar[:, None] >= ar[None, :]                    # [T, Tk]
        key_block = ar // BLOCK

        # branch 2: topk block-sparse
        keymask = sel[:, :, key_block]                         # [G, T, Tk]
        smask = keymask & causal[None]
        ss = jnp.where(smask[:, None], s, NEG)
        p2 = jax.nn.softmax(ss, axis=-1).astype(bf16)
        sparse_out = jnp.einsum('ghts,sgd->tghd', p2, vb,
                                preferred_element_type=f32).reshape(T, HQ, D)

        # branch 3: sliding window
        delta = ar[:, None] - ar[None, :]
        wmask = (delta >= 0) & (delta <= WINDOW)
        ws = jnp.where(wmask[None, None], s, NEG)
        p3 = jax.nn.softmax(ws, axis=-1).astype(bf16)
        slide_out = jnp.einsum('ghts,sgd->tghd', p3, vb,
                               preferred_element_type=f32).reshape(T, HQ, D)

        out = (gate[:, 0, None, None] * comp_out +
               gate[:, 1, None, None] * sparse_out +
               gate[:, 2, None, None] * slide_out)             # [T, HQ, D]
        o = (out.reshape(T, HQ * D).astype(bf16) @ Wo.T.astype(bf16)).astype(f32)
        return o[None]

    return jax.jit(fwd)


def _get_fn_and_device():
    if "fn" in _STATE:
        return _STATE["fn"], _STATE["dev"]
    import jax
    dev = None
    for d in jax.devices():
        if d.platform != "cpu":
            dev = d
            break
    if dev is None:
        dev = jax.devices()[0]
    fn = _build()
    _STATE["fn"] = fn
    _STATE["dev"] = dev
    return fn, dev


def kernel(hidden_states, Wq, Wk, Wv, Wo, Wgate, compress_key, compress_value):
    import jax
    fn, dev = _get_fn_and_device()
    args = [np.asarray(a, dtype=np.float32) for a in
            (hidden_states, Wq, Wk, Wv, Wo, Wgate, compress_key, compress_value)]
    with jax.default_device(dev):
        out = fn(*args)
        out = np.asarray(out)
    return out.astype(np.float32)
